# revision 21
# baseline (speedup 1.0000x reference)
# Trainium2 Bass kernel for nn_Decoder_52733608460677.
#
# Seq2seq greedy decoder: 2-layer GRU + dot-product attention + vocab
# projection, 15 steps, batch 128, vocab 32000, hidden 1024.
#
# Distribution: pure data-parallel over batch — each of the 8 NeuronCores
# runs 16 batch rows end-to-end against the full (replicated) model, with
# ZERO cross-core collectives (collectives cost ~1 ms each on this
# runtime, so any per-step cooperation loses).
#
# All matmuls are fp32 (reference argmax margins are ~1e-5; bf16/fp16
# would flip greedy tokens and wreck whole trajectories). The M=16
# batch-row matmuls are packed 4 K-chunks at a time into the PE array's
# four 32-wide column groups (tile_position) and the partial sums are
# folded with a small [128,16] selection matmul, recovering ~4x PE
# throughput. Weights stream from HBM each step and overlap compute.
import numpy as np

V, E, H, L, B, S, T = 32000, 512, 1024, 2, 128, 128, 15
NC = 8
BL = B // NC          # 16 batch rows per core
HC = H // 128         # 8 hidden chunks
VCH = 500             # dense2 vocab chunk (psum free dim)
NVC = V // VCH        # 64 vocab chunks
G3 = 3 * H            # 3072 gate width
NG = G3 // 512        # 6 gate n-chunks

_CACHE = {}


def _build():
    import concourse.bass as bass
    import concourse.bacc as bacc
    import concourse.mybir as mybir
    import concourse.tile as tile
    from concourse.masks import make_identity

    f32 = mybir.dt.float32
    u32 = mybir.dt.uint32
    Alu = mybir.AluOpType
    Act = mybir.ActivationFunctionType
    Ax = mybir.AxisListType

    nc = bacc.Bacc("TRN2", target_bir_lowering=False, debug=False, num_devices=NC)

    def din(name, shape, dt=f32):
        return nc.dram_tensor(name, shape, dt, kind="ExternalInput").ap()

    emb_d = din("emb", [V, E])
    eT_d = din("encT", [128, BL * HC * 128])      # [h', (b, hc, s)]
    eN_d = din("encN", [128, BL * H])             # [s, (b, h)]
    wih0_d = din("wih0T", [128, 4 * G3])          # [k', (kc, 3H)]
    whh0_d = din("whh0T", [128, 8 * G3])
    wih1_d = din("wih1T", [128, 8 * G3])
    whh1_d = din("whh1T", [128, 8 * G3])
    w1_d = din("w1T", [128, 16 * H])              # [k', (kc, H)]
    w2_d = din("w2T", [128, NVC * HC * VCH])      # [k', (v, kc, s)]
    b2_d = din("b2c", [1, V])
    sel_d = din("sel", [128, BL])                 # fold matrix: sel[32j+b, b]=1
    brow_d = din("brow", [BL, 1])                 # b*NVC (chunk-gather row base)
    h0n_d = din("h0nat", [BL, H])
    h1n_d = din("h1nat", [BL, H])
    h0T_d = din("h0T", [128, HC * BL])
    h1T_d = din("h1T", [128, HC * BL])

    logp_d = nc.dram_tensor("logp", [BL, T, V], f32, kind="ExternalOutput").ap()
    hT_d = nc.dram_tensor("hT", [2, 128, HC * BL], f32, kind="ExternalOutput").ap()

    with tile.TileContext(nc) as tc:
        import contextlib
        ctx = contextlib.ExitStack()
        with ctx:
            res = ctx.enter_context(tc.tile_pool(name="res", bufs=1))
            sb = ctx.enter_context(tc.tile_pool(name="sb", bufs=2))
            w2p = ctx.enter_context(tc.tile_pool(name="w2p", bufs=3))
            strm = ctx.enter_context(tc.tile_pool(name="strm", bufs=2))
            ps = ctx.enter_context(tc.tile_pool(name="ps", bufs=2, space="PSUM"))
            psg = ctx.enter_context(tc.tile_pool(name="psg", bufs=2, space="PSUM"))
            dr = ctx.enter_context(tc.tile_pool(name="dr", bufs=2, space="DRAM"))

            # ---- small resident tensors ----
            ident = res.tile([128, 128], f32)
            make_identity(nc, ident[:])
            selt = res.tile([128, BL], f32)
            nc.sync.dma_start(selt[:], sel_d[:])
            browt = res.tile([BL, 1], f32)
            nc.sync.dma_start(browt[:], brow_d[:])
            ones1 = res.tile([1, BL], f32)
            nc.gpsimd.memset(ones1[:], 1.0)

            tok = res.tile([BL, 1], u32)
            nc.gpsimd.memset(tok[:], 0)
            h0n = res.tile([BL, H], f32, name="h0n_init")
            h1n = res.tile([BL, H], f32, name="h1n_init")
            h0T = res.tile([128, HC * BL], f32, name="h0T_init")
            h1T = res.tile([128, HC * BL], f32, name="h1T_init")
            nc.sync.dma_start(h0n[:], h0n_d[:])
            nc.sync.dma_start(h1n[:], h1n_d[:])
            nc.sync.dma_start(h0T[:], h0T_d[:])
            nc.sync.dma_start(h1T[:], h1T_d[:])

            for z in range(2):
                gpz = ps.tile([128, 512], f32, name=f"gpz_{z}", tag="gp")
                nc.vector.memset(gpz[:], 0.0)

            # HBM scratch for the full local logits [BL, V] each step
            lgsc = dr.tile([BL * NVC, VCH], f32, name="lgscratch", tag="lgsc", bufs=2)

            def transpose128(src_ap, dst_ap, tag):
                p, n = src_ap.shape
                tp = psg.tile([128, 128], f32, name=f"tp_{tag}", tag="tp")
                nc.tensor.transpose(tp[:n, :p], src_ap, ident[:p, :p])
                nc.vector.tensor_copy(dst_ap, tp[:n, :p])

            def packed_mm(out_sb_ap, lhs_cols, rhs_tiles, nfree, tag, extra_rhs=None):
                """out[BL, nfree] = sum_kc lhs_cols[kc].T @ rhs_tiles[kc]
                (each lhs [128, BL]); K-chunks packed 4 per round into the
                PE col-groups, partials folded with the selt matmul.
                extra_rhs: optional [1, nfree] bias added via ones1."""
                nkc = len(lhs_cols)
                nr = (nkc + 3) // 4
                gp = ps.tile([128, nfree], f32, name=f"gp_{tag}", tag="gp")
                for r in range(nr):
                    for j in range(min(4, nkc - r * 4)):
                        kc = r * 4 + j
                        nc.tensor.matmul(
                            gp[32 * j:32 * j + BL, :], lhs_cols[kc], rhs_tiles[kc],
                            start=(r == 0), stop=(r == nr - 1),
                            tile_position=(0, 32 * j))
                gps = sb.tile([128, nfree], f32, name=f"gps_{tag}", tag=f"gps{nfree}")
                nc.vector.tensor_copy(gps[:], gp[:])
                cmb = psg.tile([BL, nfree], f32, name=f"cmb_{tag}", tag="cmb")
                nc.tensor.matmul(cmb[:], selt[:], gps[:],
                                 start=True, stop=(extra_rhs is None))
                if extra_rhs is not None:
                    nc.tensor.matmul(cmb[:], ones1[:], extra_rhs,
                                     start=False, stop=True)
                nc.vector.tensor_copy(out_sb_ap, cmb[:])

            def load_w4(dram, nkc_total, kcbase, nch, nfree, lname):
                wg = strm.tile([128, 4 * 512], f32, name=f"wg_{lname}", tag="wg", bufs=3)
                nc.sync.dma_start(
                    wg[:].rearrange("p (kc n) -> p kc n", kc=4)[:, :, :nfree],
                    dram[:].rearrange("p (kc g) -> p kc g", kc=nkc_total)[
                        :, kcbase:kcbase + 4, nch * nfree:(nch + 1) * nfree])
                return [wg[:, k * 512:k * 512 + nfree] for k in range(4)]

            for t in range(T):
                # ---------- embedding gather + transpose ----------
                x = sb.tile([BL, E], f32, name=f"x_{t}", tag="x")
                nc.gpsimd.indirect_dma_start(
                    out=x[:], out_offset=None, in_=emb_d[:],
                    in_offset=bass.IndirectOffsetOnAxis(ap=tok[:, :1], axis=0),
                )
                xT = sb.tile([128, 4 * BL], f32, name=f"xT_{t}", tag="xT")
                for j in range(4):
                    transpose128(x[:, j * 128:(j + 1) * 128],
                                 xT[:, j * BL:(j + 1) * BL], f"x{t}_{j}")

                # ---------- GRU (both layers) ----------
                def gru_layer(xT_cols, hT_cols, wih_dr, whh_dr, nkci, hprev, lname):
                    gi = sb.tile([BL, G3], f32, name=f"gi_{lname}", tag="gi", bufs=1)
                    gh = sb.tile([BL, G3], f32, name=f"gh_{lname}", tag="gh", bufs=1)
                    for nch in range(NG):
                        rhs = load_w4(wih_dr, nkci, 0, nch, 512, f"i_{lname}_{nch}_0")
                        if nkci == 8:
                            rhs += load_w4(wih_dr, nkci, 4, nch, 512, f"i_{lname}_{nch}_4")
                        packed_mm(gi[:, nch * 512:(nch + 1) * 512],
                                  xT_cols, rhs, 512, f"gi_{lname}_{nch}")
                        rhs = (load_w4(whh_dr, 8, 0, nch, 512, f"h_{lname}_{nch}_0")
                               + load_w4(whh_dr, 8, 4, nch, 512, f"h_{lname}_{nch}_4"))
                        packed_mm(gh[:, nch * 512:(nch + 1) * 512],
                                  hT_cols, rhs, 512, f"gh_{lname}_{nch}")
                    rz = sb.tile([BL, 2 * H], f32, name=f"rz_{lname}", tag="rz", bufs=1)
                    nc.vector.tensor_add(rz[:], gi[:, :2 * H], gh[:, :2 * H])
                    nc.scalar.activation(rz[:], rz[:], Act.Sigmoid)
                    nt = sb.tile([BL, H], f32, name=f"nt_{lname}", tag="nt", bufs=1)
                    nc.vector.tensor_mul(nt[:], rz[:, :H], gh[:, 2 * H:])
                    nc.vector.tensor_add(nt[:], nt[:], gi[:, 2 * H:])
                    nc.scalar.activation(nt[:], nt[:], Act.Tanh)
                    hnew = sb.tile([BL, H], f32, name=f"hn_{lname}", tag="hnew", bufs=3)
                    nc.vector.tensor_sub(hnew[:], hprev[:], nt[:])
                    nc.vector.tensor_mul(hnew[:], rz[:, H:], hnew[:])
                    nc.vector.tensor_add(hnew[:], hnew[:], nt[:])
                    hTn = sb.tile([128, HC * BL], f32, name=f"hT_{lname}", tag="hTn", bufs=3)
                    for hc in range(HC):
                        transpose128(hnew[:, hc * 128:(hc + 1) * 128],
                                     hTn[:, hc * BL:(hc + 1) * BL], f"hT_{lname}_{hc}")
                    return hnew, hTn

                xT_cols = [xT[:, kc * BL:(kc + 1) * BL] for kc in range(4)]
                h0T_cols = [h0T[:, kc * BL:(kc + 1) * BL] for kc in range(HC)]
                h1T_cols = [h1T[:, kc * BL:(kc + 1) * BL] for kc in range(HC)]
                h0n, h0T = gru_layer(xT_cols, h0T_cols, wih0_d, whh0_d, 4, h0n, f"l0_{t}")
                h0T_cols = [h0T[:, kc * BL:(kc + 1) * BL] for kc in range(HC)]
                h1n, h1T = gru_layer(h0T_cols, h1T_cols, wih1_d, whh1_d, 8, h1n, f"l1_{t}")

                # ---------- attention (groups of 4 rows via col-tiling) ----------
                ctxT = sb.tile([128, HC * BL], f32, name=f"ctxT_{t}", tag="ctxT")
                for g in range(4):
                    sc4_ps = ps.tile([128, 128], f32, name=f"sc4_{t}_{g}", tag="pwork", bufs=2)
                    nc.vector.memset(sc4_ps[:], 0.0)
                    for hc in range(HC):
                        eThc = strm.tile([128, 4 * 128], f32, name=f"eThc_{t}_{g}_{hc}", tag="eThc")
                        nc.sync.dma_start(
                            eThc[:].rearrange("p (j s) -> p j s", j=4),
                            eT_d[:].rearrange("p (b hc s) -> p b hc s", b=BL, hc=HC)[:, g * 4:(g + 1) * 4, hc, :])
                        for j in range(4):
                            b = g * 4 + j
                            nc.tensor.matmul(
                                sc4_ps[32 * j:32 * j + 1, :],
                                h1T[:, hc * BL + b: hc * BL + b + 1],
                                eThc[:, j * 128:(j + 1) * 128],
                                start=(hc == 0), stop=(hc == 7),
                                tile_position=(0, 32 * j))
                    scs = sb.tile([128, 128], f32, name=f"scs_{t}_{g}", tag="scs")
                    nc.vector.tensor_copy(scs[:], sc4_ps[:])
                    smax = sb.tile([128, 1], f32, name=f"smax_{t}_{g}", tag="smax")
                    nc.vector.tensor_reduce(smax[:], scs[:], Ax.X, Alu.max)
                    nsmax = sb.tile([128, 1], f32, name=f"nsmax_{t}_{g}", tag="nsmax")
                    nc.vector.tensor_scalar_mul(nsmax[:], smax[:], -1.0)
                    watt = sb.tile([128, 128], f32, name=f"watt_{t}_{g}", tag="watt")
                    ssum = sb.tile([128, 1], f32, name=f"ssum_{t}_{g}", tag="ssum")
                    nc.scalar.activation(watt[:], scs[:], Act.Exp,
                                         bias=nsmax[:, :1], accum_out=ssum[:, :1])
                    rsum = sb.tile([128, 1], f32, name=f"rsum_{t}_{g}", tag="rsum")
                    nc.vector.reciprocal(rsum[:], ssum[:])
                    nc.vector.tensor_scalar_mul(watt[:], watt[:], rsum[:, :1])
                    wT4 = sb.tile([128, 128], f32, name=f"wT4_{t}_{g}", tag="wT4")
                    transpose128(watt[:], wT4[:], f"w{t}_{g}")
                    for half in range(2):
                        eNg = strm.tile([128, 4 * 512], f32, name=f"eNg_{t}_{g}_{half}", tag="eNg")
                        nc.sync.dma_start(
                            eNg[:].rearrange("p (j s) -> p j s", j=4),
                            eN_d[:].rearrange("p (b hf s) -> p b hf s", b=BL, hf=2)[:, g * 4:(g + 1) * 4, half, :])
                        cx_ps = ps.tile([128, 512], f32, name=f"cx_{t}_{g}_{half}", tag="pwork", bufs=2)
                        for j in range(4):
                            nc.tensor.matmul(
                                cx_ps[32 * j:32 * j + 1, :],
                                wT4[:, 32 * j:32 * j + 1],
                                eNg[:, j * 512:(j + 1) * 512],
                                start=True, stop=True,
                                tile_position=(0, 32 * j))
                        cx_sb = sb.tile([128, 512], f32, name=f"cxs_{t}_{g}_{half}", tag="cxsb")
                        nc.vector.tensor_copy(cx_sb[:], cx_ps[:])
                        for hcl in range(4):
                            hcg = half * 4 + hcl
                            tpc = psg.tile([128, 128], f32, name=f"tpc_{t}_{g}_{hcg}", tag="tp")
                            nc.tensor.transpose(tpc[:], cx_sb[:, hcl * 128:(hcl + 1) * 128], ident[:])
                            nc.vector.tensor_copy(
                                ctxT[:, hcg * BL + g * 4: hcg * BL + g * 4 + 4],
                                tpc[:].rearrange("p (a c) -> p a c", c=32)[:, :, 0])

                # ---------- dense1 (col-packed, 16 k-chunks) ----------
                osb = sb.tile([BL, H], f32, name=f"osb_{t}", tag="osb")
                cat_cols = ([h1T[:, kc * BL:(kc + 1) * BL] for kc in range(HC)]
                            + [ctxT[:, kc * BL:(kc + 1) * BL] for kc in range(HC)])
                for half in range(2):
                    rhs = []
                    for kcb in range(0, 16, 4):
                        rhs += load_w4(w1_d, 16, kcb, half, 512, f"w1_{t}_{half}_{kcb}")
                    packed_mm(osb[:, half * 512:(half + 1) * 512],
                              cat_cols, rhs, 512, f"d1_{t}_{half}")
                nc.scalar.activation(osb[:], osb[:], Act.Tanh)
                oT = sb.tile([128, HC * BL], f32, name=f"oT_{t}", tag="oT")
                for hc in range(HC):
                    transpose128(osb[:, hc * 128:(hc + 1) * 128],
                                 oT[:, hc * BL:(hc + 1) * BL], f"o{t}_{hc}")

                # ---------- dense2: full vocab; logits -> HBM scratch ----------
                cm = sb.tile([BL, NVC], f32, name=f"cm_{t}", tag="cm")
                oT_cols = [oT[:, kc * BL:(kc + 1) * BL] for kc in range(HC)]
                for v in range(NVC):
                    rhs = []
                    for kcb in range(2):
                        w2g = w2p.tile([128, 4 * VCH], f32, name=f"w2g_{t}_{v}_{kcb}", tag="w2g", bufs=4)
                        nc.sync.dma_start(
                            w2g[:].rearrange("p (kc s) -> p kc s", kc=4),
                            w2_d[:].rearrange("p (v kc s) -> p v kc s", v=NVC, kc=HC)[
                                :, v, kcb * 4:(kcb + 1) * 4, :])
                        rhs += [w2g[:, k * VCH:(k + 1) * VCH] for k in range(4)]
                    b2v = strm.tile([1, VCH], f32, name=f"b2v_{t}_{v}", tag="b2v")
                    nc.sync.dma_start(b2v[:], b2_d[:, v * VCH:(v + 1) * VCH])
                    lch = sb.tile([BL, VCH], f32, name=f"lch_{t}_{v}", tag="lch")
                    packed_mm(lch[:],
                              oT_cols, rhs,
                              VCH, f"d2_{t}_{v}", extra_rhs=b2v[:1, :])
                    nc.sync.dma_start(
                        lgsc[:].rearrange("(b v) s -> b v s", v=NVC)[:, v, :], lch[:])
                    nc.vector.tensor_reduce(cm[:, v:v + 1], lch[:], Ax.X, Alu.max)

                # ---------- greedy argmax (2-level, core-local) ----------
                cmx = sb.tile([BL, 8], f32, name=f"cmx_{t}", tag="cmx")
                cmi = sb.tile([BL, 8], u32, name=f"cmi_{t}", tag="cmi")
                nc.vector.max_with_indices(cmx[:], cmi[:], cm[:])
                cif = sb.tile([BL, 1], f32, name=f"cif_{t}", tag="cif")
                nc.vector.tensor_copy(cif[:], cmi[:, :1])
                grow = sb.tile([BL, 1], f32, name=f"grow_{t}", tag="grow")
                nc.vector.tensor_add(grow[:], cif[:], browt[:])
                growi = sb.tile([BL, 1], u32, name=f"growi_{t}", tag="growi")
                nc.vector.tensor_copy(growi[:], grow[:])
                wch = sb.tile([BL, VCH], f32, name=f"wch_{t}", tag="wch")
                nc.gpsimd.indirect_dma_start(
                    out=wch[:], out_offset=None, in_=lgsc[:],
                    in_offset=bass.IndirectOffsetOnAxis(ap=growi[:, :1], axis=0),
                )
                wmx = sb.tile([BL, 8], f32, name=f"wmx_{t}", tag="wmx")
                wmi = sb.tile([BL, 8], u32, name=f"wmi_{t}", tag="wmi")
                nc.vector.max_with_indices(wmx[:], wmi[:], wch[:])
                wif = sb.tile([BL, 1], f32, name=f"wif_{t}", tag="wif")
                nc.vector.tensor_copy(wif[:], wmi[:, :1])
                gidx = sb.tile([BL, 1], f32, name=f"gidx_{t}", tag="gidx")
                nc.vector.tensor_scalar(out=gidx[:], in0=cif[:], scalar1=float(VCH),
                                        scalar2=None, op0=Alu.mult)
                nc.vector.tensor_add(gidx[:], gidx[:], wif[:])
                tok_new = res.tile([BL, 1], u32, name=f"tok_{t}", bufs=2, tag="tokv")
                nc.vector.tensor_copy(tok_new[:], gidx[:])
                tok = tok_new

                # ---------- log-softmax correction (off critical path) ----------
                gmax = sb.tile([BL, 1], f32, name=f"gmax_{t}", tag="gmax")
                nc.vector.tensor_copy(gmax[:], cmx[:, :1])
                ngmax = sb.tile([BL, 1], f32, name=f"ngmax_{t}", tag="ngmax")
                nc.vector.tensor_scalar_mul(ngmax[:], gmax[:], -1.0)
                se = sb.tile([BL, NVC], f32, name=f"se_{t}", tag="se")
                for v in range(NVC):
                    rch = sb.tile([BL, VCH], f32, name=f"rch_{t}_{v}", tag="rch")
                    nc.sync.dma_start(
                        rch[:], lgsc[:].rearrange("(b v) s -> b v s", v=NVC)[:, v, :])
                    ech = sb.tile([BL, VCH], f32, name=f"ech_{t}_{v}", tag="ech")
                    nc.scalar.activation(ech[:], rch[:], Act.Exp,
                                         bias=ngmax[:, :1], accum_out=se[:, v:v + 1])
                sume = sb.tile([BL, 1], f32, name=f"sume_{t}", tag="sume")
                nc.vector.tensor_reduce(sume[:], se[:], Ax.X, Alu.add)
                lns = sb.tile([BL, 1], f32, name=f"lns_{t}", tag="lns")
                nc.scalar.activation(lns[:], sume[:], Act.Ln)
                ncorr = sb.tile([BL, 1], f32, name=f"ncorr_{t}", tag="ncorr")
                nc.vector.tensor_add(ncorr[:], lns[:], gmax[:])
                nc.vector.tensor_scalar_mul(ncorr[:], ncorr[:], -1.0)
                for v in range(NVC):
                    qch = sb.tile([BL, VCH], f32, name=f"qch_{t}_{v}", tag="qch")
                    nc.sync.dma_start(
                        qch[:], lgsc[:].rearrange("(b v) s -> b v s", v=NVC)[:, v, :])
                    pch = sb.tile([BL, VCH], f32, name=f"pch_{t}_{v}", tag="pch")
                    nc.vector.tensor_scalar_add(pch[:], qch[:], ncorr[:, :1])
                    nc.sync.dma_start(logp_d[:, t, v * VCH:(v + 1) * VCH], pch[:])

            nc.sync.dma_start(hT_d[0], h0T[:])
            nc.sync.dma_start(hT_d[1], h1T[:])
    nc.compile()
    return nc


def _prep_inputs(inputs):
    f = np.float32
    emb = np.ascontiguousarray(inputs["emb"], dtype=f)
    enc = np.asarray(inputs["encoder_outputs"], dtype=f)
    ehs = np.asarray(inputs["encoder_hidden_state"], dtype=f)
    W1 = np.asarray(inputs["W1"], dtype=f)
    W2 = np.asarray(inputs["W2"], dtype=f)
    b2 = np.asarray(inputs["b2"], dtype=f)

    def kchunked(wT, nk):
        K, N = wT.shape
        assert K == nk * 128
        return np.ascontiguousarray(
            wT.reshape(nk, 128, N).transpose(1, 0, 2).reshape(128, nk * N), dtype=f)

    wih0T = kchunked(inputs["Wih0"].T.astype(f), 4)      # [512,3072] -> [128, 4*3072]
    whh0T = kchunked(inputs["Whh0"].T.astype(f), 8)
    wih1T = kchunked(inputs["Wih1"].T.astype(f), 8)
    whh1T = kchunked(inputs["Whh1"].T.astype(f), 8)
    w1T = kchunked(W1.T.astype(f), 16)                   # [2048,1024] -> [128, 16*1024]
    w2T = np.ascontiguousarray(
        W2.T.reshape(HC, 128, NVC, VCH).transpose(1, 2, 0, 3).reshape(128, NVC * HC * VCH),
        dtype=f)
    b2c = b2.reshape(1, V).astype(f)
    sel = np.zeros((128, BL), f)
    for j in range(4):
        sel[32 * j + np.arange(BL), np.arange(BL)] = 1.0
    brow = (np.arange(BL, dtype=f) * NVC).reshape(BL, 1)

    in_maps = []
    for c in range(NC):
        m = {"emb": emb, "wih0T": wih0T, "whh0T": whh0T, "wih1T": wih1T,
             "whh1T": whh1T, "w1T": w1T, "w2T": w2T, "b2c": b2c,
             "sel": sel, "brow": brow}
        encc = enc[c * BL:(c + 1) * BL]                  # [16, 128, 1024]
        m["encT"] = np.ascontiguousarray(
            encc.reshape(BL, S, HC, 128).transpose(3, 0, 2, 1).reshape(128, BL * HC * 128), dtype=f)
        m["encN"] = np.ascontiguousarray(
            encc.transpose(1, 0, 2).reshape(128, BL * H), dtype=f)
        m["h0nat"] = np.ascontiguousarray(ehs[0][c * BL:(c + 1) * BL], dtype=f)
        m["h1nat"] = np.ascontiguousarray(ehs[1][c * BL:(c + 1) * BL], dtype=f)
        m["h0T"] = np.ascontiguousarray(
            ehs[0][c * BL:(c + 1) * BL].T.reshape(HC, 128, BL).transpose(1, 0, 2).reshape(128, HC * BL), dtype=f)
        m["h1T"] = np.ascontiguousarray(
            ehs[1][c * BL:(c + 1) * BL].T.reshape(HC, 128, BL).transpose(1, 0, 2).reshape(128, HC * BL), dtype=f)
        in_maps.append(m)
    return in_maps


def kernel(**inputs):
    from concourse.bass_utils import run_bass_kernel_spmd

    if "nc" not in _CACHE:
        _CACHE["nc"] = _build()
    nc = _CACHE["nc"]
    in_maps = _prep_inputs(inputs)
    res = run_bass_kernel_spmd(nc, in_maps, core_ids=list(range(NC)))
    _CACHE["last_res"] = res
    outs = res.results
    dec = np.concatenate([outs[c]["logp"] for c in range(NC)], axis=0)  # [128, 15, 32000]
    hf = np.zeros((2, B, H), np.float32)
    for c in range(NC):
        hT = outs[c]["hT"].reshape(2, 128, HC, BL)
        hf[:, c * BL:(c + 1) * BL, :] = hT.transpose(0, 3, 2, 1).reshape(2, BL, H)
    return dec, hf
